# revision 62
# baseline (speedup 1.0000x reference)
"""Trainium2 Bass kernel for nn_Block_51883204936056 (dense_cnn pyramid block).

Data-parallel over batch (8 cores, one batch element each). Per core:
  - 5 iterations of {conv3x3 + bias + residual + clip} on three pyramid levels
  - cross-scale fusion (avg-pool / nearest-upsample + 1x1 conv) -> 3 outputs

Implementation notes:
  - Images stored in SBUF as overlapped 128-row blocks (stride BS=126) of the
    zero-padded image P (P row 0 = top pad, P rows 1..H = image, P row H+1 =
    bottom pad). Rotated convention so DVE ops start at partition 0:
      block t, partition p (p=0..126)  <->  P row BS*t + 1 + p (image row BS*t+p)
      block t, partition 127           <->  P row BS*t       (image row BS*t-1)
    Columns have 1 zero pad on each side.
  - conv3x3 on TensorE: 3 banded fp32r matmuls (one per kernel column) with
    column-shifted rhs windows accumulating in PSUM -> full 2D conv. The band
    matrix maps rhs partition k -> out partition m; the wrapped halo row
    (partition 127) feeds out row m=0.  Valid psum partitions: 0..125.
  - Epilogue on VectorE: scalar_tensor_tensor (conv+bias+BU) then clip
    (tensor_scalar max/min) written back to the image buffer partitions 0..125.
  - Halo cells (partition 126 = next block's first row, partition 127) are
    refreshed by tiny SBUF->SBUF DMAs after each iteration.
  - Fusion: row pooling/upsampling via matmuls with host-built pool/repeat
    matrices (1x1-conv weights folded in); column pooling via strided rhs APs,
    column upsampling via step-0 repeat APs.  The "identity" term of each 1x1
    conv is fused into the PSUM-evicting scalar_tensor_tensor.
"""

import math

import numpy as np

import concourse.bass as bass
import concourse.tile as tile
from concourse import bacc, mybir
from concourse.ap import AP
from concourse.bass_utils import run_bass_kernel_spmd

F32 = mybir.dt.float32
F32R = mybir.dt.float32r
ADD = mybir.AluOpType.add
MULT = mybir.AluOpType.mult
MAXOP = mybir.AluOpType.max
MINOP = mybir.AluOpType.min


# --------------------------------------------------------------------------- #
# Custom fused DVE op: out = clip(in0 + bias + in1, -1, 1) in one pass
# --------------------------------------------------------------------------- #

_EPILOGUE_OP = None


def _register_epilogue_op():
    global _EPILOGUE_OP
    if _EPILOGUE_OP is not None:
        return _EPILOGUE_OP
    from concourse import dve_ops as dops
    from concourse.dve_spec import Spec, Src0, Src1, C0, C1, C2, maxx, minn, lower
    from concourse.dve_spec import _has_src1
    from concourse.dve_uop import DveOpSpec

    name = "CONV_EPILOGUE_ANT"
    for op in dops.OPS:
        if op.name == name:
            _EPILOGUE_OP = op
            return op
    spec = Spec(
        body=minn(maxx((Src0 + C0) + Src1, C1), C2),
        reference=lambda in0, in1, s0, s1, imm2: np.minimum(
            np.maximum(in0.astype(np.float32) + s0 + in1, s1), imm2),
    )
    row = max(dops._SUB_OPCODE_FOR_NAME.values()) + 1
    assert row < 0x20
    dops._SUB_OPCODE_FOR_NAME[name] = row
    shas = {}
    for ver in ("v3", "v4"):
        try:
            shas[ver] = DveOpSpec(name=name, opcode=row, uops=lower(spec, ver=ver),
                                  rd1_en=_has_src1(spec)).sha(ver)
        except Exception:
            pass
    op = dops.DveOp(name, spec, subdim=False, uops_sha=shas)
    dops.OPS.append(op)
    dops.CUSTOM_DVE_SPECS[name] = spec
    _EPILOGUE_OP = op
    return op

NCORES = 8
NITER = 5
BS = 126  # valid rows per overlapped 128-row block
B = 8

LEVELS = {
    "x": dict(H=512, W=512, img="x", bu="BU", conv="convAx", bias="biasx"),
    "down": dict(H=256, W=256, img="downSample", bu="BUdown", conv="convAd", bias="biasd"),
    "up": dict(H=1024, W=1024, img="upSample", bu="BUup", conv="convAu", bias="biasu"),
}

# terms: (kind, src_level, factor, channel); channel indexes conv1x1_*[0, ch, 0, 0]
# "rep" with factor=1 is the identity term (1x1-conv weight folded into matrix)
OUTS = {
    "x_out": dict(H=512, W=512, w="conv1x1_x",
                  terms=[("pool", "up", 2, 0), ("stt", "x", 1, 1), ("rep", "down", 2, 2)]),
    "down_out": dict(H=256, W=256, w="conv1x1_d",
                     terms=[("pool", "up", 4, 0), ("pool", "x", 2, 1), ("stt", "down", 1, 2)]),
    "up_out": dict(H=1024, W=1024, w="conv1x1_u",
                   terms=[("stt", "up", 1, 0), ("rep", "x", 2, 1), ("rep", "down", 4, 2)]),
}

CONST_COLS = {"bias_x": 0, "bias_down": 1, "bias_up": 2,
              "stt_x_out": 3, "stt_down_out": 4, "stt_up_out": 5}


def nb_of(H):
    return math.ceil(H / BS)


def mcount(H, t):
    return min(BS, H - BS * t)


# --------------------------------------------------------------------------- #
# Plan: matrix registry + fusion matmul schedule (shared by trace + host build)
# --------------------------------------------------------------------------- #

class Plan:
    def __init__(self):
        self.wids = {}
        self.mats = []
        self._mat_idx = {}
        self.conv_mat = {}
        self.fusion = {}

    def wid(self, ref):
        if ref not in self.wids:
            self.wids[ref] = len(self.wids)
        return self.wids[ref]

    def mat(self, entries):
        key = frozenset(entries)
        if key not in self._mat_idx:
            self._mat_idx[key] = len(self.mats)
            self.mats.append(key)
        return self._mat_idx[key]


def _pool_entries(t, factor, H_src, wid):
    """avg-pool rows: out row r=BS*t+m <- src rows factor*r + a."""
    by_s = {}
    scale = 1.0 / (factor * factor)
    nbs = nb_of(H_src)
    for m in range(BS):
        r = BS * t + m
        for a in range(factor):
            g = factor * r + a
            s = g // BS
            if s >= nbs:
                continue
            p = g - BS * s
            by_s.setdefault(s, set()).add((p, m, wid, scale))
    return by_s


def _rep_entries(t, factor, H_src, wid):
    """nearest-up rows: out row r=BS*t+m <- src row r // factor."""
    by_s = {}
    nbs = nb_of(H_src)
    for m in range(BS):
        r = BS * t + m
        g = r // factor
        s = g // BS
        if s >= nbs:
            continue
        p = g - BS * s
        by_s.setdefault(s, set()).add((p, m, wid, 1.0))
    return by_s


def make_plan():
    plan = Plan()
    # conv band matrices: out partition m (out image row BS*t+m) taps
    # P rows BS*t + m + a (a=0..2); P row BS*t+Q sits at partition Q-1
    # (or 127 when Q==0).
    for lvl in LEVELS:
        for bcol in range(3):
            entries = set()
            for m in range(BS):
                for a in range(3):
                    q = m + a
                    k = 127 if q == 0 else q - 1
                    entries.add((k, m, plan.wid(("conv", lvl, a, bcol)), 1.0))
            plan.conv_mat[(lvl, bcol)] = plan.mat(entries)

    for oname, osp in OUTS.items():
        Ho, Wo = osp["H"], osp["W"]
        per_block = []
        for t in range(nb_of(Ho)):
            for chunk in range(math.ceil(Wo / 512)):
                Wc = min(512, Wo - 512 * chunk)
                mms = []
                stt_term = None
                for kind, src, factor, ch in osp["terms"]:
                    wid = plan.wid(("c1x1", oname, ch))
                    Hs = LEVELS[src]["H"]
                    if kind == "stt":
                        stt_term = src
                        continue
                    if kind == "pool":
                        by_s = _pool_entries(t, factor, Hs, wid)
                        for s, ent in sorted(by_s.items()):
                            mi = plan.mat(ent)
                            for phi in range(factor):
                                c0 = 1 + factor * 512 * chunk + phi
                                mms.append((mi, src, s, c0, [[factor, Wc]]))
                    else:
                        by_s = _rep_entries(t, factor, Hs, wid)
                        for s, ent in sorted(by_s.items()):
                            mi = plan.mat(ent)
                            c0 = 1 + (512 * chunk) // factor
                            fd = [[1, Wc]] if factor == 1 else [[1, Wc // factor], [0, factor]]
                            mms.append((mi, src, s, c0, fd))
                per_block.append(dict(t=t, chunk=chunk, Wc=Wc, mms=mms, stt=stt_term))
        plan.fusion[oname] = per_block
    return plan


# --------------------------------------------------------------------------- #
# Device program
# --------------------------------------------------------------------------- #

def _buf_ap(handle, tot, block, F, p0, pn, c0, cn, freedims=None, dtype=None):
    free = freedims if freedims is not None else [[1, cn]]
    ap = AP(handle, p0 * tot + block * F + c0, [[tot, pn]] + free)
    if dtype is not None:
        ap = ap.bitcast(dtype)
    return ap


def build_program(plan):
    nc = bacc.Bacc("TRN2", target_bir_lowering=False, debug=False,
                   enable_asserts=False, num_devices=NCORES)
    nmat = len(plan.mats)

    dram = {}
    for lvl, sp in LEVELS.items():
        dram[lvl] = nc.dram_tensor(sp["img"], [sp["H"], sp["W"]], F32, kind="ExternalInput")
        dram["bu_" + lvl] = nc.dram_tensor(sp["bu"], [sp["H"], sp["W"]], F32, kind="ExternalInput")
    dram["mats"] = nc.dram_tensor("mats", [128, nmat * 128], F32, kind="ExternalInput")
    dram["consts"] = nc.dram_tensor("consts", [128, 8], F32, kind="ExternalInput")
    for oname, osp in OUTS.items():
        dram[oname] = nc.dram_tensor(oname, [osp["H"], osp["W"]], F32, kind="ExternalOutput")

    sb = {}
    tots = {}
    for lvl, sp in LEVELS.items():
        F = sp["W"] + 2
        nb = nb_of(sp["H"])
        tots[lvl] = nb * F
        tots["bu_" + lvl] = nb * sp["W"]
        sb[lvl] = nc.alloc_sbuf_tensor(f"buf_{lvl}", [128, nb * F], F32)
        sb["bu_" + lvl] = nc.alloc_sbuf_tensor(f"bubuf_{lvl}", [128, nb * sp["W"]], F32)
    sb["mats"] = nc.alloc_sbuf_tensor("mats_sb", [128, nmat * 128], F32)
    sb["consts"] = nc.alloc_sbuf_tensor("consts_sb", [128, 8], F32)

    def img_ap(lvl, block, p0, pn, c0, cn, freedims=None, rounded=True):
        return _buf_ap(sb[lvl], tots[lvl], block, LEVELS[lvl]["W"] + 2, p0, pn, c0, cn,
                       freedims, F32R if rounded else None)

    def bu_ap(lvl, block, pn, cn, c0=0):
        return _buf_ap(sb["bu_" + lvl], tots["bu_" + lvl], block, LEVELS[lvl]["W"],
                       0, pn, c0, cn)

    def mat_ap(mi):
        return sb["mats"].ap()[:, mi * 128:(mi + 1) * 128].bitcast(F32R)

    def const_ap(col, pn):
        return sb["consts"].ap()[0:pn, col:col + 1]

    ep_op = _register_epilogue_op()
    dma_engines = None  # set inside context

    with tile.TileContext(nc) as tc:
        with tc.tile_pool(name="work", bufs=8) as work_pool, \
             tc.tile_pool(name="psum", bufs=4, space="PSUM") as psum_pool, \
             tc.tile_pool(name="psum2", bufs=2, space="PSUM") as psum2_pool:

            # ---- init: zero only pad columns, junk rows, and block-0 top pad ----
            for lvl, sp in LEVELS.items():
                Ww = sp["W"]
                F = Ww + 2
                nb = nb_of(sp["H"])
                tot = tots[lvl]
                # pad columns 0 and W+1 of every block
                nc.gpsimd.memset(AP(sb[lvl], 0, [[tot, 128], [F, nb], [Ww + 1, 2]]), 0.0)
                # last block: zero everything below the loaded rows (loads
                # overwrite partitions 0..nrow afterwards)
                nc.gpsimd.memset(
                    AP(sb[lvl], (nb - 1) * F, [[tot, 127], [1, F]]), 0.0)
                # block 0 partition 127 = P row 0 (top pad); start partition 96
                # (engine APs must start at 0/32/64/96), loads rewrite 96..126
                nc.gpsimd.memset(AP(sb[lvl], 96 * tot, [[tot, 32], [1, F]]), 0.0)

            # conv band matrices first (they gate the first matmul); the
            # fusion matrices are loaded after all image/BU data below.
            ncm = 9 * 128
            nc.sync.dma_start(sb["mats"].ap()[:, 0:ncm].bitcast(F32R),
                              dram["mats"].ap()[:, 0:ncm].bitcast(F32R))
            nc.scalar.dma_start(sb["consts"].ap(), dram["consts"].ap())

            # ---- image + BU loads, issue order == need order:
            # x imgs, down imgs, BU_x, BU_down, up imgs, BU_up
            def load_level_images(lvl, skip_first=False):
                sp = LEVELS[lvl]
                Hh, Ww = sp["H"], sp["W"]
                F = Ww + 2
                nb = nb_of(Hh)
                tot = tots[lvl]
                b0 = 1 if skip_first else 0
                nc.sync.dma_start(
                    AP(sb[lvl], b0 * F + 1, [[tot, 127], [F, nb - 1 - b0], [1, Ww]]).bitcast(F32R),
                    AP(dram[lvl], b0 * BS * Ww, [[Ww, 127], [BS * Ww, nb - 1 - b0], [1, Ww]]).bitcast(F32R))
                t = nb - 1
                nrow = Hh - BS * t
                nc.sync.dma_start(
                    img_ap(lvl, t, 0, nrow, 1, Ww),
                    dram[lvl].ap()[BS * t:BS * t + nrow, :].bitcast(F32R))
                nc.sync.dma_start(
                    AP(sb[lvl], 127 * tot + F + 1, [[tot, 1], [F, nb - 1], [1, Ww]]).bitcast(F32R),
                    AP(dram[lvl], (BS - 1) * Ww, [[Ww, 1], [BS * Ww, nb - 1], [1, Ww]]).bitcast(F32R))

            def load_level_bu(lvl):
                sp = LEVELS[lvl]
                Hh, Ww = sp["H"], sp["W"]
                nb = nb_of(Hh)
                bt = tots["bu_" + lvl]
                nc.sync.dma_start(
                    AP(sb["bu_" + lvl], 0, [[bt, BS], [Ww, nb - 1], [1, Ww]]),
                    AP(dram["bu_" + lvl], 0, [[Ww, BS], [BS * Ww, nb - 1], [1, Ww]]))
                mc = mcount(Hh, nb - 1)
                nc.sync.dma_start(
                    bu_ap(lvl, nb - 1, mc, Ww),
                    dram["bu_" + lvl].ap()[BS * (nb - 1):BS * (nb - 1) + mc, :])

            # need-order loads: mats, x, BU_x, down, BU_down, up (per block),
            # then BU_up (split) - see timeline analysis
            nc.sync.dma_start(
                img_ap("x", 0, 0, 127, 1, 512),
                dram["x"].ap()[0:127, :].bitcast(F32R))
            load_level_images("x", skip_first=True)
            load_level_bu("x")
            load_level_images("down")
            load_level_bu("down")
            upt = tots["up"]
            nbu = nb_of(1024)
            for t in range(nbu):
                nrow = min(127, 1024 - BS * t)
                nc.sync.dma_start(
                    img_ap("up", t, 0, nrow, 1, 1024),
                    dram["up"].ap()[BS * t:BS * t + nrow, :].bitcast(F32R))
            nc.sync.dma_start(
                AP(sb["up"], 127 * upt + 1026 + 1, [[upt, 1], [1026, nbu - 1], [1, 1024]]).bitcast(F32R),
                AP(dram["up"], (BS - 1) * 1024, [[1024, 1], [BS * 1024, nbu - 1], [1, 1024]]).bitcast(F32R))
            but = tots["bu_up"]
            nc.sync.dma_start(
                AP(sb["bu_up"], 0, [[but, BS], [1024, 4], [1, 1024]]),
                AP(dram["bu_up"], 0, [[1024, BS], [BS * 1024, 4], [1, 1024]]))
            nc.sync.dma_start(
                AP(sb["bu_up"], 4 * 1024, [[but, BS], [1024, 4], [1, 1024]]),
                AP(dram["bu_up"], 4 * BS * 1024, [[1024, BS], [BS * 1024, 4], [1, 1024]]))
            mcl = mcount(1024, nbu - 1)
            nc.sync.dma_start(
                bu_ap("up", nbu - 1, mcl, 1024),
                dram["bu_up"].ap()[BS * (nbu - 1):BS * (nbu - 1) + mcl, :])

            # fusion matrices (needed only after the iterations)
            if nmat > 9:
                nc.sync.dma_start(sb["mats"].ap()[:, ncm:].bitcast(F32R),
                                    dram["mats"].ap()[:, ncm:].bitcast(F32R))

            bias_col = {"x": CONST_COLS["bias_x"], "down": CONST_COLS["bias_down"],
                        "up": CONST_COLS["bias_up"]}

            def level_iteration(lvl, halos=True, blocks=None, first=False):
                sp = LEVELS[lvl]
                Hh, Ww = sp["H"], sp["W"]
                F = Ww + 2
                nb = nb_of(Hh)
                tot = tots[lvl]
                nchunk = math.ceil(Ww / 512)
                for t in (range(nb) if blocks is None else blocks):
                    mc = mcount(Hh, t)
                    if Ww > 512:
                        ps = psum2_pool.tile([128, 1024], F32, tag="ps1024")
                    else:
                        ps = psum_pool.tile([128, 512], F32, tag="ps512")
                    for c in range(nchunk):
                        Wc = min(512, Ww - 512 * c)
                        for bcol in range(3):
                            nc.tensor.matmul(
                                ps[:, 512 * c:512 * c + Wc],
                                mat_ap(plan.conv_mat[(lvl, bcol)]),
                                img_ap(lvl, t, 0, 128, 512 * c + bcol, Wc),
                                start=(bcol == 0), stop=(bcol == 2))
                    nc.vector._custom_dve(
                        ep_op,
                        out=img_ap(lvl, t, 0, mc, 1, Ww),
                        in0=ps[0:mc, 0:Ww],
                        in1=bu_ap(lvl, t, mc, Ww),
                        s0=const_ap(bias_col[lvl], mc),
                        s1=-1.0, imm2=1.0)
                if not halos:
                    return
                # halo refresh, one batched DMA per direction:
                #   cell (t,126) <- block t+1 partition 0   (t = 0..nb-2)
                #   cell (t,127) <- block t-1 partition 125 (t = 1..nb-1)
                nc.sync.dma_start(
                    AP(sb[lvl], 126 * tot + 1, [[tot, 1], [F, nb - 1], [1, Ww]]).bitcast(F32R),
                    AP(sb[lvl], 0 * tot + F + 1, [[tot, 1], [F, nb - 1], [1, Ww]]).bitcast(F32R))
                nc.scalar.dma_start(
                    AP(sb[lvl], 127 * tot + F + 1, [[tot, 1], [F, nb - 1], [1, Ww]]).bitcast(F32R),
                    AP(sb[lvl], 125 * tot + 1, [[tot, 1], [F, nb - 1], [1, Ww]]).bitcast(F32R))

            # ---- 5 iterations; `up` runs 2 iterations behind x/down so its
            # (large, DMA-paced) first-iteration input load overlaps compute ----
            for it in range(NITER):
                level_iteration("x", halos=it < NITER - 1)
                level_iteration("down", halos=it < NITER - 1)
                level_iteration("up", halos=it < NITER - 1)

            # ---- fusion: pool/rep terms as matmuls; identity term fused into
            # the PSUM-evicting scalar_tensor_tensor on VectorE ----
            stt_col = {"x_out": CONST_COLS["stt_x_out"],
                       "down_out": CONST_COLS["stt_down_out"],
                       "up_out": CONST_COLS["stt_up_out"]}
            out_engines = [nc.gpsimd, nc.sync, nc.scalar]
            oi = 0
            fusion_units = []
            ups = [("up_out", b) for b in plan.fusion["up_out"]]
            xs = [("x_out", b) for b in plan.fusion["x_out"]]
            downs = [("down_out", b) for b in plan.fusion["down_out"]]
            # weave x blocks into the up stream (1 x per 3 ups), downs last
            xi = 0
            for i, u in enumerate(ups):
                fusion_units.append(u)
                if i % 3 == 2 and xi < len(xs):
                    fusion_units.append(xs[xi]); xi += 1
            fusion_units.extend(xs[xi:])
            fusion_units.extend(downs)
            if True:
                for oname, blk in fusion_units:
                    osp = OUTS[oname]
                    Ho = osp["H"]
                    t, chunk, Wc, mms = blk["t"], blk["chunk"], blk["Wc"], blk["mms"]
                    src_stt = blk["stt"]
                    mc = mcount(Ho, t)
                    ps = psum_pool.tile([128, 512], F32, tag="ps512")
                    for i, (mi, src, s, c0, freedims) in enumerate(mms):
                        nc.tensor.matmul(
                            ps[:, 0:Wc], sb["mats"].ap()[0:126, mi * 128:(mi + 1) * 128].bitcast(F32R),
                            img_ap(src, s, 0, 126, c0, Wc, freedims=freedims),
                            start=(i == 0), stop=(i == len(mms) - 1))
                    ot = work_pool.tile([128, Wc], F32, tag=f"out{Wc}")
                    nc.vector.scalar_tensor_tensor(
                        ot[0:mc, 0:Wc],
                        img_ap(src_stt, t, 0, mc, 1 + 512 * chunk, Wc, rounded=False),
                        const_ap(stt_col[oname], mc),
                        ps[0:mc, 0:Wc],
                        MULT, ADD)
                    out_engines[oi % len(out_engines)].dma_start(
                        dram[oname].ap()[BS * t:BS * t + mc, 512 * chunk:512 * chunk + Wc],
                        ot[0:mc, 0:Wc])
                    oi += 1

    nc.compile()
    return nc


# --------------------------------------------------------------------------- #
# Host side
# --------------------------------------------------------------------------- #

def build_host_arrays(plan, inputs):
    wvals = np.zeros(max(len(plan.wids), 1), np.float32)
    for ref, idx in plan.wids.items():
        if ref[0] == "conv":
            _, lvl, a, bcol = ref
            wvals[idx] = np.asarray(inputs[LEVELS[lvl]["conv"]], np.float32)[0, 0, a, bcol]
        else:
            _, oname, ch = ref
            wvals[idx] = np.asarray(inputs[OUTS[oname]["w"]], np.float32)[0, ch, 0, 0]

    nmat = len(plan.mats)
    mats = np.zeros((128, nmat * 128), np.float32)
    for i, entries in enumerate(plan.mats):
        for (p, m, wid, scale) in entries:
            mats[p, i * 128 + m] += wvals[wid] * scale

    consts = np.zeros((128, 8), np.float32)
    consts[:, CONST_COLS["bias_x"]] = np.asarray(inputs["biasx"], np.float32)[0]
    consts[:, CONST_COLS["bias_down"]] = np.asarray(inputs["biasd"], np.float32)[0]
    consts[:, CONST_COLS["bias_up"]] = np.asarray(inputs["biasu"], np.float32)[0]
    consts[:, CONST_COLS["stt_x_out"]] = np.asarray(inputs["conv1x1_x"], np.float32)[0, 1, 0, 0]
    consts[:, CONST_COLS["stt_down_out"]] = np.asarray(inputs["conv1x1_d"], np.float32)[0, 2, 0, 0]
    consts[:, CONST_COLS["stt_up_out"]] = np.asarray(inputs["conv1x1_u"], np.float32)[0, 0, 0, 0]
    return mats, consts


_CACHE = {}
LAST_RESULTS = None


def kernel(**inputs):
    global LAST_RESULTS
    if "prog" not in _CACHE:
        plan = make_plan()
        _CACHE["plan"] = plan
        _CACHE["prog"] = build_program(plan)
    plan, nc = _CACHE["plan"], _CACHE["prog"]

    mats, consts = build_host_arrays(plan, inputs)

    in_maps = []
    for c in range(NCORES):
        m = {"mats": mats, "consts": consts}
        for lvl, sp in LEVELS.items():
            m[sp["img"]] = np.ascontiguousarray(
                np.asarray(inputs[sp["img"]], np.float32)[c, 0])
            m[sp["bu"]] = np.ascontiguousarray(
                np.asarray(inputs[sp["bu"]], np.float32)[c, 0])
        in_maps.append(m)

    res = run_bass_kernel_spmd(nc, in_maps, list(range(NCORES)))
    LAST_RESULTS = res

    outs = []
    for oname, osp in OUTS.items():
        arr = np.stack([res.results[c][oname] for c in range(NCORES)])
        outs.append(arr.reshape(B, 1, osp["H"], osp["W"]).astype(np.float32))
    return tuple(outs)


# revision 64
# speedup vs baseline: 1.0465x; 1.0465x over previous
"""Trainium2 Bass kernel for nn_Block_51883204936056 (dense_cnn pyramid block).

Data-parallel over batch (8 cores, one batch element each). Per core:
  - 5 iterations of {conv3x3 + bias + residual + clip} on three pyramid levels
  - cross-scale fusion (avg-pool / nearest-upsample + 1x1 conv) -> 3 outputs

Implementation notes:
  - Images stored in SBUF as overlapped 128-row blocks (stride BS=126) of the
    zero-padded image P (P row 0 = top pad, P rows 1..H = image, P row H+1 =
    bottom pad). Rotated convention so DVE ops start at partition 0:
      block t, partition p (p=0..126)  <->  P row BS*t + 1 + p (image row BS*t+p)
      block t, partition 127           <->  P row BS*t       (image row BS*t-1)
    Columns have 1 zero pad on each side.
  - conv3x3 on TensorE: 3 banded fp32r matmuls (one per kernel column) with
    column-shifted rhs windows accumulating in PSUM -> full 2D conv. The band
    matrix maps rhs partition k -> out partition m; the wrapped halo row
    (partition 127) feeds out row m=0.  Valid psum partitions: 0..125.
  - Epilogue on VectorE: scalar_tensor_tensor (conv+bias+BU) then clip
    (tensor_scalar max/min) written back to the image buffer partitions 0..125.
  - Halo cells (partition 126 = next block's first row, partition 127) are
    refreshed by tiny SBUF->SBUF DMAs after each iteration.
  - Fusion: row pooling/upsampling via matmuls with host-built pool/repeat
    matrices (1x1-conv weights folded in); column pooling via strided rhs APs,
    column upsampling via step-0 repeat APs.  The "identity" term of each 1x1
    conv is fused into the PSUM-evicting scalar_tensor_tensor.
"""

import math

import numpy as np

import concourse.bass as bass
import concourse.tile as tile
from concourse import bacc, mybir
from concourse.ap import AP
from concourse.bass_utils import run_bass_kernel_spmd

F32 = mybir.dt.float32
F32R = mybir.dt.float32r
ADD = mybir.AluOpType.add
MULT = mybir.AluOpType.mult
MAXOP = mybir.AluOpType.max
MINOP = mybir.AluOpType.min


# --------------------------------------------------------------------------- #
# Custom fused DVE op: out = clip(in0 + bias + in1, -1, 1) in one pass
# --------------------------------------------------------------------------- #

_EPILOGUE_OP = None


def _register_epilogue_op():
    global _EPILOGUE_OP
    if _EPILOGUE_OP is not None:
        return _EPILOGUE_OP
    from concourse import dve_ops as dops
    from concourse.dve_spec import Spec, Src0, Src1, C0, C1, C2, maxx, minn, lower
    from concourse.dve_spec import _has_src1
    from concourse.dve_uop import DveOpSpec

    name = "CONV_EPILOGUE_ANT"
    for op in dops.OPS:
        if op.name == name:
            _EPILOGUE_OP = op
            return op
    spec = Spec(
        body=minn(maxx((Src0 + C0) + Src1, C1), C2),
        reference=lambda in0, in1, s0, s1, imm2: np.minimum(
            np.maximum(in0.astype(np.float32) + s0 + in1, s1), imm2),
    )
    row = max(dops._SUB_OPCODE_FOR_NAME.values()) + 1
    assert row < 0x20
    dops._SUB_OPCODE_FOR_NAME[name] = row
    shas = {}
    for ver in ("v3", "v4"):
        try:
            shas[ver] = DveOpSpec(name=name, opcode=row, uops=lower(spec, ver=ver),
                                  rd1_en=_has_src1(spec)).sha(ver)
        except Exception:
            pass
    op = dops.DveOp(name, spec, subdim=False, uops_sha=shas)
    dops.OPS.append(op)
    dops.CUSTOM_DVE_SPECS[name] = spec
    _EPILOGUE_OP = op
    return op

NCORES = 8
NITER = 5
BS = 126  # valid rows per overlapped 128-row block
B = 8

LEVELS = {
    "x": dict(H=512, W=512, img="x", bu="BU", conv="convAx", bias="biasx"),
    "down": dict(H=256, W=256, img="downSample", bu="BUdown", conv="convAd", bias="biasd"),
    "up": dict(H=1024, W=1024, img="upSample", bu="BUup", conv="convAu", bias="biasu"),
}

# terms: (kind, src_level, factor, channel); channel indexes conv1x1_*[0, ch, 0, 0]
# "rep" with factor=1 is the identity term (1x1-conv weight folded into matrix)
OUTS = {
    "x_out": dict(H=512, W=512, w="conv1x1_x",
                  terms=[("pool", "up", 2, 0), ("stt", "x", 1, 1), ("rep", "down", 2, 2)]),
    "down_out": dict(H=256, W=256, w="conv1x1_d",
                     terms=[("pool", "up", 4, 0), ("pool", "x", 2, 1), ("stt", "down", 1, 2)]),
    "up_out": dict(H=1024, W=1024, w="conv1x1_u",
                   terms=[("stt", "up", 1, 0), ("rep", "x", 2, 1), ("rep", "down", 4, 2)]),
}

CONST_COLS = {"bias_x": 0, "bias_down": 1, "bias_up": 2,
              "stt_x_out": 3, "stt_down_out": 4, "stt_up_out": 5}


def nb_of(H):
    return math.ceil(H / BS)


def mcount(H, t):
    return min(BS, H - BS * t)


# --------------------------------------------------------------------------- #
# Plan: matrix registry + fusion matmul schedule (shared by trace + host build)
# --------------------------------------------------------------------------- #

class Plan:
    def __init__(self):
        self.wids = {}
        self.mats = []
        self._mat_idx = {}
        self.conv_mat = {}
        self.fusion = {}

    def wid(self, ref):
        if ref not in self.wids:
            self.wids[ref] = len(self.wids)
        return self.wids[ref]

    def mat(self, entries):
        key = frozenset(entries)
        if key not in self._mat_idx:
            self._mat_idx[key] = len(self.mats)
            self.mats.append(key)
        return self._mat_idx[key]


def _pool_entries(t, factor, H_src, wid):
    """avg-pool rows: out row r=BS*t+m <- src rows factor*r + a."""
    by_s = {}
    scale = 1.0 / (factor * factor)
    nbs = nb_of(H_src)
    for m in range(BS):
        r = BS * t + m
        for a in range(factor):
            g = factor * r + a
            s = g // BS
            if s >= nbs:
                continue
            p = g - BS * s
            by_s.setdefault(s, set()).add((p, m, wid, scale))
    return by_s


def _rep_entries(t, factor, H_src, wid):
    """nearest-up rows: out row r=BS*t+m <- src row r // factor."""
    by_s = {}
    nbs = nb_of(H_src)
    for m in range(BS):
        r = BS * t + m
        g = r // factor
        s = g // BS
        if s >= nbs:
            continue
        p = g - BS * s
        by_s.setdefault(s, set()).add((p, m, wid, 1.0))
    return by_s


def make_plan():
    plan = Plan()
    # conv band matrices: out partition m (out image row BS*t+m) taps
    # P rows BS*t + m + a (a=0..2); P row BS*t+Q sits at partition Q-1
    # (or 127 when Q==0).
    for lvl in LEVELS:
        for bcol in range(3):
            entries = set()
            for m in range(BS):
                for a in range(3):
                    q = m + a
                    k = 127 if q == 0 else q - 1
                    entries.add((k, m, plan.wid(("conv", lvl, a, bcol)), 1.0))
            plan.conv_mat[(lvl, bcol)] = plan.mat(entries)

    for oname, osp in OUTS.items():
        Ho, Wo = osp["H"], osp["W"]
        per_block = []
        for t in range(nb_of(Ho)):
            for chunk in range(math.ceil(Wo / 512)):
                Wc = min(512, Wo - 512 * chunk)
                mms = []
                stt_term = None
                for kind, src, factor, ch in osp["terms"]:
                    wid = plan.wid(("c1x1", oname, ch))
                    Hs = LEVELS[src]["H"]
                    if kind == "stt":
                        stt_term = src
                        continue
                    if kind == "pool":
                        by_s = _pool_entries(t, factor, Hs, wid)
                        for s, ent in sorted(by_s.items()):
                            mi = plan.mat(ent)
                            for phi in range(factor):
                                c0 = 1 + factor * 512 * chunk + phi
                                mms.append((mi, src, s, c0, [[factor, Wc]]))
                    else:
                        by_s = _rep_entries(t, factor, Hs, wid)
                        for s, ent in sorted(by_s.items()):
                            mi = plan.mat(ent)
                            c0 = 1 + (512 * chunk) // factor
                            fd = [[1, Wc]] if factor == 1 else [[1, Wc // factor], [0, factor]]
                            mms.append((mi, src, s, c0, fd))
                per_block.append(dict(t=t, chunk=chunk, Wc=Wc, mms=mms, stt=stt_term))
        plan.fusion[oname] = per_block
    return plan


# --------------------------------------------------------------------------- #
# Device program
# --------------------------------------------------------------------------- #

def _buf_ap(handle, tot, block, F, p0, pn, c0, cn, freedims=None, dtype=None):
    free = freedims if freedims is not None else [[1, cn]]
    ap = AP(handle, p0 * tot + block * F + c0, [[tot, pn]] + free)
    if dtype is not None:
        ap = ap.bitcast(dtype)
    return ap


def build_program(plan):
    nc = bacc.Bacc("TRN2", target_bir_lowering=False, debug=False,
                   enable_asserts=False, num_devices=NCORES)
    nmat = len(plan.mats)

    dram = {}
    for lvl, sp in LEVELS.items():
        dram[lvl] = nc.dram_tensor(sp["img"], [sp["H"], sp["W"]], F32, kind="ExternalInput")
        dram["bu_" + lvl] = nc.dram_tensor(sp["bu"], [sp["H"], sp["W"]], F32, kind="ExternalInput")
    dram["mats"] = nc.dram_tensor("mats", [128, nmat * 128], F32, kind="ExternalInput")
    dram["consts"] = nc.dram_tensor("consts", [128, 8], F32, kind="ExternalInput")
    for oname, osp in OUTS.items():
        dram[oname] = nc.dram_tensor(oname, [osp["H"], osp["W"]], F32, kind="ExternalOutput")

    sb = {}
    tots = {}
    for lvl, sp in LEVELS.items():
        F = sp["W"] + 2
        nb = nb_of(sp["H"])
        tots[lvl] = nb * F
        tots["bu_" + lvl] = nb * sp["W"]
        sb[lvl] = nc.alloc_sbuf_tensor(f"buf_{lvl}", [128, nb * F], F32)
        sb["bu_" + lvl] = nc.alloc_sbuf_tensor(f"bubuf_{lvl}", [128, nb * sp["W"]], F32)
    sb["mats"] = nc.alloc_sbuf_tensor("mats_sb", [128, nmat * 128], F32)
    sb["consts"] = nc.alloc_sbuf_tensor("consts_sb", [128, 8], F32)

    def img_ap(lvl, block, p0, pn, c0, cn, freedims=None, rounded=True):
        return _buf_ap(sb[lvl], tots[lvl], block, LEVELS[lvl]["W"] + 2, p0, pn, c0, cn,
                       freedims, F32R if rounded else None)

    def bu_ap(lvl, block, pn, cn, c0=0):
        return _buf_ap(sb["bu_" + lvl], tots["bu_" + lvl], block, LEVELS[lvl]["W"],
                       0, pn, c0, cn)

    def mat_ap(mi):
        return sb["mats"].ap()[:, mi * 128:(mi + 1) * 128].bitcast(F32R)

    def const_ap(col, pn):
        return sb["consts"].ap()[0:pn, col:col + 1]

    ep_op = _register_epilogue_op()
    dma_engines = None  # set inside context

    with tile.TileContext(nc) as tc:
        with tc.tile_pool(name="work", bufs=8) as work_pool, \
             tc.tile_pool(name="psum", bufs=4, space="PSUM") as psum_pool, \
             tc.tile_pool(name="psum2", bufs=2, space="PSUM") as psum2_pool:

            # ---- init: zero only pad columns, junk rows, and block-0 top pad ----
            for lvl, sp in LEVELS.items():
                Ww = sp["W"]
                F = Ww + 2
                nb = nb_of(sp["H"])
                tot = tots[lvl]
                # pad columns 0 and W+1 of every block
                nc.gpsimd.memset(AP(sb[lvl], 0, [[tot, 128], [F, nb], [Ww + 1, 2]]), 0.0)
                # last block: zero everything below the loaded rows (loads
                # overwrite partitions 0..nrow afterwards)
                nc.gpsimd.memset(
                    AP(sb[lvl], (nb - 1) * F, [[tot, 127], [1, F]]), 0.0)
                # block 0 partition 127 = P row 0 (top pad); start partition 96
                # (engine APs must start at 0/32/64/96), loads rewrite 96..126
                nc.gpsimd.memset(AP(sb[lvl], 96 * tot, [[tot, 32], [1, F]]), 0.0)

            # conv band matrices first (they gate the first matmul); the
            # fusion matrices are loaded after all image/BU data below.
            ncm = 9 * 128
            nc.sync.dma_start(sb["mats"].ap()[:, 0:3 * 128].bitcast(F32R),
                              dram["mats"].ap()[:, 0:3 * 128].bitcast(F32R))
            nc.sync.dma_start(sb["mats"].ap()[:, 3 * 128:ncm].bitcast(F32R),
                              dram["mats"].ap()[:, 3 * 128:ncm].bitcast(F32R))
            nc.scalar.dma_start(sb["consts"].ap(), dram["consts"].ap())

            # ---- image + BU loads, issue order == need order:
            # x imgs, down imgs, BU_x, BU_down, up imgs, BU_up
            def load_level_images(lvl, skip_first=False):
                sp = LEVELS[lvl]
                Hh, Ww = sp["H"], sp["W"]
                F = Ww + 2
                nb = nb_of(Hh)
                tot = tots[lvl]
                b0 = 1 if skip_first else 0
                nc.sync.dma_start(
                    AP(sb[lvl], b0 * F + 1, [[tot, 127], [F, nb - 1 - b0], [1, Ww]]).bitcast(F32R),
                    AP(dram[lvl], b0 * BS * Ww, [[Ww, 127], [BS * Ww, nb - 1 - b0], [1, Ww]]).bitcast(F32R))
                t = nb - 1
                nrow = Hh - BS * t
                nc.sync.dma_start(
                    img_ap(lvl, t, 0, nrow, 1, Ww),
                    dram[lvl].ap()[BS * t:BS * t + nrow, :].bitcast(F32R))
                nc.sync.dma_start(
                    AP(sb[lvl], 127 * tot + F + 1, [[tot, 1], [F, nb - 1], [1, Ww]]).bitcast(F32R),
                    AP(dram[lvl], (BS - 1) * Ww, [[Ww, 1], [BS * Ww, nb - 1], [1, Ww]]).bitcast(F32R))

            def load_level_bu(lvl):
                sp = LEVELS[lvl]
                Hh, Ww = sp["H"], sp["W"]
                nb = nb_of(Hh)
                bt = tots["bu_" + lvl]
                nc.sync.dma_start(
                    AP(sb["bu_" + lvl], 0, [[bt, BS], [Ww, nb - 1], [1, Ww]]),
                    AP(dram["bu_" + lvl], 0, [[Ww, BS], [BS * Ww, nb - 1], [1, Ww]]))
                mc = mcount(Hh, nb - 1)
                nc.sync.dma_start(
                    bu_ap(lvl, nb - 1, mc, Ww),
                    dram["bu_" + lvl].ap()[BS * (nb - 1):BS * (nb - 1) + mc, :])

            # need-order loads: mats, x, BU_x, down, BU_down, up (per block),
            # then BU_up (split) - see timeline analysis
            nc.sync.dma_start(
                img_ap("x", 0, 0, 127, 1, 512),
                dram["x"].ap()[0:127, :].bitcast(F32R))
            load_level_images("x", skip_first=True)
            load_level_bu("x")
            load_level_images("down")
            load_level_bu("down")
            upt = tots["up"]
            nbu = nb_of(1024)
            for t in range(nbu):
                nrow = min(127, 1024 - BS * t)
                nc.sync.dma_start(
                    img_ap("up", t, 0, nrow, 1, 1024),
                    dram["up"].ap()[BS * t:BS * t + nrow, :].bitcast(F32R))
            nc.sync.dma_start(
                AP(sb["up"], 127 * upt + 1026 + 1, [[upt, 1], [1026, nbu - 1], [1, 1024]]).bitcast(F32R),
                AP(dram["up"], (BS - 1) * 1024, [[1024, 1], [BS * 1024, nbu - 1], [1, 1024]]).bitcast(F32R))
            but = tots["bu_up"]
            nc.sync.dma_start(
                AP(sb["bu_up"], 0, [[but, BS], [1024, 4], [1, 1024]]),
                AP(dram["bu_up"], 0, [[1024, BS], [BS * 1024, 4], [1, 1024]]))
            nc.sync.dma_start(
                AP(sb["bu_up"], 4 * 1024, [[but, BS], [1024, 4], [1, 1024]]),
                AP(dram["bu_up"], 4 * BS * 1024, [[1024, BS], [BS * 1024, 4], [1, 1024]]))
            mcl = mcount(1024, nbu - 1)
            nc.sync.dma_start(
                bu_ap("up", nbu - 1, mcl, 1024),
                dram["bu_up"].ap()[BS * (nbu - 1):BS * (nbu - 1) + mcl, :])

            # fusion matrices (needed only after the iterations)
            if nmat > 9:
                nc.sync.dma_start(sb["mats"].ap()[:, ncm:].bitcast(F32R),
                                    dram["mats"].ap()[:, ncm:].bitcast(F32R))

            bias_col = {"x": CONST_COLS["bias_x"], "down": CONST_COLS["bias_down"],
                        "up": CONST_COLS["bias_up"]}

            def level_iteration(lvl, halos=True, blocks=None, first=False):
                sp = LEVELS[lvl]
                Hh, Ww = sp["H"], sp["W"]
                F = Ww + 2
                nb = nb_of(Hh)
                tot = tots[lvl]
                nchunk = math.ceil(Ww / 512)
                for t in (range(nb) if blocks is None else blocks):
                    mc = mcount(Hh, t)
                    if Ww > 512:
                        ps = psum2_pool.tile([128, 1024], F32, tag="ps1024")
                    else:
                        ps = psum_pool.tile([128, 512], F32, tag="ps512")
                    for c in range(nchunk):
                        Wc = min(512, Ww - 512 * c)
                        for bcol in range(3):
                            nc.tensor.matmul(
                                ps[:, 512 * c:512 * c + Wc],
                                mat_ap(plan.conv_mat[(lvl, bcol)]),
                                img_ap(lvl, t, 0, 128, 512 * c + bcol, Wc),
                                start=(bcol == 0), stop=(bcol == 2))
                    nc.vector._custom_dve(
                        ep_op,
                        out=img_ap(lvl, t, 0, mc, 1, Ww),
                        in0=ps[0:mc, 0:Ww],
                        in1=bu_ap(lvl, t, mc, Ww),
                        s0=const_ap(bias_col[lvl], mc),
                        s1=-1.0, imm2=1.0)
                if not halos:
                    return
                # halo refresh, one batched DMA per direction:
                #   cell (t,126) <- block t+1 partition 0   (t = 0..nb-2)
                #   cell (t,127) <- block t-1 partition 125 (t = 1..nb-1)
                nc.sync.dma_start(
                    AP(sb[lvl], 126 * tot + 1, [[tot, 1], [F, nb - 1], [1, Ww]]).bitcast(F32R),
                    AP(sb[lvl], 0 * tot + F + 1, [[tot, 1], [F, nb - 1], [1, Ww]]).bitcast(F32R))
                nc.scalar.dma_start(
                    AP(sb[lvl], 127 * tot + F + 1, [[tot, 1], [F, nb - 1], [1, Ww]]).bitcast(F32R),
                    AP(sb[lvl], 125 * tot + 1, [[tot, 1], [F, nb - 1], [1, Ww]]).bitcast(F32R))

            # ---- 5 iterations; `up` runs 2 iterations behind x/down so its
            # (large, DMA-paced) first-iteration input load overlaps compute ----
            for it in range(NITER):
                level_iteration("x", halos=it < NITER - 1)
                level_iteration("down", halos=it < NITER - 1)
                level_iteration("up", halos=it < NITER - 1)

            # ---- fusion: pool/rep terms as matmuls; identity term fused into
            # the PSUM-evicting scalar_tensor_tensor on VectorE ----
            stt_col = {"x_out": CONST_COLS["stt_x_out"],
                       "down_out": CONST_COLS["stt_down_out"],
                       "up_out": CONST_COLS["stt_up_out"]}
            out_engines = [nc.gpsimd, nc.sync, nc.scalar]
            oi = 0
            upb = plan.fusion["up_out"]
            up_units = [("up_out", (upb[2 * i], upb[2 * i + 1]))
                        for i in range(len(upb) // 2)]
            xs = [("x_out", b) for b in plan.fusion["x_out"]]
            downs = [("down_out", b) for b in plan.fusion["down_out"]]
            fusion_units = []
            xi = 0
            for i, u in enumerate(up_units):
                fusion_units.append(u)
                if i % 2 == 1 and xi < len(xs):
                    fusion_units.append(xs[xi]); xi += 1
            fusion_units.extend(xs[xi:])
            fusion_units.extend(downs)
            for oname, blk in fusion_units:
                osp = OUTS[oname]
                Ho = osp["H"]
                if oname == "up_out":
                    b0, b1 = blk
                    t = b0["t"]
                    mc = mcount(Ho, t)
                    ps = psum2_pool.tile([128, 1024], F32, tag="ps1024")
                    for half in (b0, b1):
                        ck, mms = half["chunk"], half["mms"]
                        for i, (mi, src, s, c0, freedims) in enumerate(mms):
                            nc.tensor.matmul(
                                ps[:, 512 * ck:512 * ck + 512],
                                sb["mats"].ap()[0:126, mi * 128:(mi + 1) * 128].bitcast(F32R),
                                img_ap(src, s, 0, 126, c0, 512, freedims=freedims),
                                start=(i == 0), stop=(i == len(mms) - 1))
                    ot = work_pool.tile([128, 1024], F32, tag="out1024")
                    nc.vector.scalar_tensor_tensor(
                        ot[0:mc, 0:1024],
                        img_ap(b0["stt"], t, 0, mc, 1, 1024, rounded=False),
                        const_ap(stt_col[oname], mc),
                        ps[0:mc, 0:1024],
                        MULT, ADD)
                    out_engines[oi % len(out_engines)].dma_start(
                        dram[oname].ap()[BS * t:BS * t + mc, :],
                        ot[0:mc, 0:1024])
                    oi += 1
                    continue
                t, chunk, Wc, mms = blk["t"], blk["chunk"], blk["Wc"], blk["mms"]
                src_stt = blk["stt"]
                mc = mcount(Ho, t)
                ps = psum_pool.tile([128, 512], F32, tag="ps512")
                for i, (mi, src, s, c0, freedims) in enumerate(mms):
                    nc.tensor.matmul(
                        ps[:, 0:Wc], sb["mats"].ap()[0:126, mi * 128:(mi + 1) * 128].bitcast(F32R),
                        img_ap(src, s, 0, 126, c0, Wc, freedims=freedims),
                        start=(i == 0), stop=(i == len(mms) - 1))
                ot = work_pool.tile([128, Wc], F32, tag=f"out{Wc}")
                nc.vector.scalar_tensor_tensor(
                    ot[0:mc, 0:Wc],
                    img_ap(src_stt, t, 0, mc, 1 + 512 * chunk, Wc, rounded=False),
                    const_ap(stt_col[oname], mc),
                    ps[0:mc, 0:Wc],
                    MULT, ADD)
                out_engines[oi % len(out_engines)].dma_start(
                    dram[oname].ap()[BS * t:BS * t + mc, 512 * chunk:512 * chunk + Wc],
                    ot[0:mc, 0:Wc])
                oi += 1

    nc.compile()
    return nc


# --------------------------------------------------------------------------- #
# Host side
# --------------------------------------------------------------------------- #

def build_host_arrays(plan, inputs):
    wvals = np.zeros(max(len(plan.wids), 1), np.float32)
    for ref, idx in plan.wids.items():
        if ref[0] == "conv":
            _, lvl, a, bcol = ref
            wvals[idx] = np.asarray(inputs[LEVELS[lvl]["conv"]], np.float32)[0, 0, a, bcol]
        else:
            _, oname, ch = ref
            wvals[idx] = np.asarray(inputs[OUTS[oname]["w"]], np.float32)[0, ch, 0, 0]

    nmat = len(plan.mats)
    mats = np.zeros((128, nmat * 128), np.float32)
    for i, entries in enumerate(plan.mats):
        for (p, m, wid, scale) in entries:
            mats[p, i * 128 + m] += wvals[wid] * scale

    consts = np.zeros((128, 8), np.float32)
    consts[:, CONST_COLS["bias_x"]] = np.asarray(inputs["biasx"], np.float32)[0]
    consts[:, CONST_COLS["bias_down"]] = np.asarray(inputs["biasd"], np.float32)[0]
    consts[:, CONST_COLS["bias_up"]] = np.asarray(inputs["biasu"], np.float32)[0]
    consts[:, CONST_COLS["stt_x_out"]] = np.asarray(inputs["conv1x1_x"], np.float32)[0, 1, 0, 0]
    consts[:, CONST_COLS["stt_down_out"]] = np.asarray(inputs["conv1x1_d"], np.float32)[0, 2, 0, 0]
    consts[:, CONST_COLS["stt_up_out"]] = np.asarray(inputs["conv1x1_u"], np.float32)[0, 0, 0, 0]
    return mats, consts


_CACHE = {}
LAST_RESULTS = None


def kernel(**inputs):
    global LAST_RESULTS
    if "prog" not in _CACHE:
        plan = make_plan()
        _CACHE["plan"] = plan
        _CACHE["prog"] = build_program(plan)
    plan, nc = _CACHE["plan"], _CACHE["prog"]

    mats, consts = build_host_arrays(plan, inputs)

    in_maps = []
    for c in range(NCORES):
        m = {"mats": mats, "consts": consts}
        for lvl, sp in LEVELS.items():
            m[sp["img"]] = np.ascontiguousarray(
                np.asarray(inputs[sp["img"]], np.float32)[c, 0])
            m[sp["bu"]] = np.ascontiguousarray(
                np.asarray(inputs[sp["bu"]], np.float32)[c, 0])
        in_maps.append(m)

    res = run_bass_kernel_spmd(nc, in_maps, list(range(NCORES)))
    LAST_RESULTS = res

    outs = []
    for oname, osp in OUTS.items():
        arr = np.stack([res.results[c][oname] for c in range(NCORES)])
        outs.append(arr.reshape(B, 1, osp["H"], osp["W"]).astype(np.float32))
    return tuple(outs)
